# revision 3
# baseline (speedup 1.0000x reference)
"""Trainium2 Bass kernel for 2D cubic Hermite interpolation (nn_CubicHermite2d).

Math: with x1 = arange(W), x2 = arange(H) (per the problem spec), the whole
op is linear in `signal`:

    result[b, r, q] = sum_{h,w} M2[h, r] * signal[b, h, w] * M1[w, q]

where M1 [W, Nx] / M2 [H, Ny] are 4-banded cubic-Hermite interpolation
matrices built on the host from xs / ys.  Queries are sorted, so contiguous
query groups have source-row bands inside a single 128-row window -> every
output block is ONE K<=128 matmul on the PE (no accumulation, no
transposes):

    step 1:  v[wlo][wp, r]  = sig[hblk, wlo:+128].T @ M2[hblk, rs:re]
    step 2:  out[b, rm, q]  = v[wlo][:, rm*128:+128].T @ M1[wlo:+128, qs:qe]

The ys (H) axis uses ALIGNED 128-row blocks: queries whose 4-row source
band crosses a 128 boundary instead use a tiny 8-row boundary signal tile
and a K=8 matmul against an 8-row packed slice of M2.  That removes the
window overlap on the load side: the signal loads as exactly 2 MB +
3x32KB boundary rows per core instead of 5 overlapping 512KB windows.
The xs axis keeps greedy overlapping windows (the v tiles are produced
on-chip, where the 5th window only costs PE cycles, not HBM bytes).

Matmuls run in float16: 1 cyc/row on the PE, FWL fast weight loads, and
half the load bytes; inputs are O(1) randn so fp16 range is a non-issue
(measured ~1.2e-3 scale-relative error vs the fp32 reference).

Loads are batched into 6 DMAs total (one [128,4,NB,W] strided DMA covers
batch 0 of all four signal blocks; one covers batches 1..NB-1), split
across the two HWDGE rings (sync + scalar) so the batch-0 critical path
lands in ~2.5us.  All output stores issue from the sync ring; the scalar
ring is DMA-free once copies start so the ACT engine's full capacity goes
to PSUM->SBUF drains.  The build software-pipelines step1(b+1) between the
two step2 halves of batch b; the last batch stores per-r-block so the
kernel tail only drains a single 256KB store.

Sharding: data-parallel over batch B=32 across 8 cores (4 batches/core).
"""

import os
import sys

import numpy as np

for _p in ("/root/.axon_site", "/root/.axon_site/_ro/trn_rl_repo",
           "/root/.axon_site/_ro/pypackages", "/opt/trn_rl_repo"):
    if os.path.isdir(_p) and _p not in sys.path:
        sys.path.append(_p)

import concourse.bass as bass
import concourse.mybir as mybir
from concourse import bacc
from concourse.bass_utils import run_bass_kernel_spmd
from concourse.tile import TileContext

# Problem shapes (hardcoded per spec)
B, H, W = 32, 512, 512
NX, NY = 1024, 1024
N_CORES = 8
NB = B // N_CORES  # batches per core

P = 128
NBLK = H // P      # aligned signal blocks (4)
NBND = NBLK - 1    # boundary tiles (3), 8 rows each
F32 = mybir.dt.float32
MM_MODE = os.environ.get("CH2D_DT", "f16")
_MM_DTS = {"f16": mybir.dt.float16, "bf16": mybir.dt.bfloat16,
           "f32r": mybir.dt.float32r, "f32": mybir.dt.float32}
# store the output as f16 and cast to f32 on host: halves the dominant
# store traffic; adds <=2^-11 relative rounding
OUT_DT16 = os.environ.get("CH2D_OUT16", "1") == "1"
VPS_BUFS = int(os.environ.get("CH2D_VPS", "2"))
OPS_BUFS = int(os.environ.get("CH2D_OPS", "4"))
N_SWDGE = int(os.environ.get("CH2D_SWDGE", "4"))
# V_COARSE: one FD=1024 copy per v tile (vs 2x FD=512)
V_COARSE = os.environ.get("CH2D_VCOARSE", "1") == "1"
WARMUP_MMS = int(os.environ.get("CH2D_WARMUP", "0"))
ACT_PREWARM = os.environ.get("CH2D_ACTWARM", "1") == "1"
# issue the bulk (batches 1..NB-1) signal load on: act (scalar HWDGE ring,
# after w1) | gp (gpsimd SWDGE queues, frees the scalar ring entirely)
BULK_ENG = os.environ.get("CH2D_BULK", "act")
STORE_SPLIT = os.environ.get("CH2D_STORE_SPLIT", "0") == "1"


def _interp_matrix(x0, u):
    """[n, Q] float64 matrix M with (y @ M) == _interp1d(y, x0, slopes, u) of
    the reference (searchsorted bucket, one-sided/averaged Hermite
    tangents)."""
    x0 = np.asarray(x0, dtype=np.float64)
    n = len(x0)
    q = len(u)
    d = np.diff(x0)  # d[j] = x0[j+1] - x0[j]
    m = np.zeros((n, q), dtype=np.float64)
    idx = np.searchsorted(x0[1:-1], u.astype(np.float64))
    dxq = d[idx]
    t = (u.astype(np.float64) - x0[idx]) / dxq
    t2, t3 = t * t, t * t * t
    h00 = 1.0 - 3.0 * t2 + 2.0 * t3
    h10 = (t - 2.0 * t2 + t3) * dxq   # multiplies m[I]
    h01 = 3.0 * t2 - 2.0 * t3
    h11 = (t3 - t2) * dxq             # multiplies m[I+1]
    for k in range(q):
        i = int(idx[k])
        m[i, k] += h00[k]
        m[i + 1, k] += h01[k]
        c = h10[k]  # m[I]: one-sided at 0, averaged interior
        if i == 0:
            m[1, k] += c / d[0]
            m[0, k] -= c / d[0]
        else:
            m[i + 1, k] += 0.5 * c / d[i]
            m[i, k] += 0.5 * c * (1.0 / d[i - 1] - 1.0 / d[i])
            m[i - 1, k] -= 0.5 * c / d[i - 1]
        c = h11[k]  # m[I+1]
        if i + 1 == n - 1:
            m[n - 1, k] += c / d[n - 2]
            m[n - 2, k] -= c / d[n - 2]
        else:
            m[i + 2, k] += 0.5 * c / d[i + 1]
            m[i + 1, k] += 0.5 * c * (1.0 / d[i] - 1.0 / d[i + 1])
            m[i, k] -= 0.5 * c / d[i]
    return m, idx.astype(np.int64)


def _make_groups(idx, n, max_size=512, bank=512):
    """Greedy contiguous query groups; each group's source rows fit a
    128-row window starting at row_lo.  Groups never cross `bank`-multiples
    in query index (PSUM bank boundary).  Returns [(q_start, q_end,
    row_lo)]."""
    qn = len(idx)
    lo = np.maximum(idx - 1, 0)
    hi = np.minimum(idx + 2, n - 1)
    groups = []
    s = 0
    while s < qn:
        row_lo = int(lo[s])
        e = s
        while e < qn:
            if hi[e] - row_lo + 1 > P:
                break
            if e - s >= max_size:
                break
            if e > s and (e % bank) == 0:
                break
            e += 1
        groups.append((s, e, min(row_lo, n - P)))
        s = e
    return groups


def _make_groups_aligned(idx, n, max_size=512, bank=512):
    """Contiguous query groups on ALIGNED 128-row blocks.  A query whose
    4-row band [idx-1, idx+2] fits block k -> (rs, re, 0, k); a band
    crossing boundary 128*(k+1) -> (rs, re, 1, k) handled by an 8-row
    boundary tile (rows 128k+124 .. 128k+131).  Groups never cross
    `bank`-multiples in query index."""
    qn = len(idx)
    lo = np.maximum(idx - 1, 0)
    hi = np.minimum(idx + 2, n - 1)

    def key(i):
        bs, be = lo[i] // P, hi[i] // P
        return (0, int(bs)) if bs == be else (1, int(bs))

    groups = []
    s = 0
    while s < qn:
        k = key(s)
        e = s
        while e < qn:
            if key(e) != k or (e > s and e % bank == 0) or e - s >= max_size:
                break
            e += 1
        groups.append((s, e, k[0], k[1]))
        s = e
    return groups


def _build_nc(g1, g2a, mm_dt):
    MM_DT = mm_dt
    OUT_DT = mybir.dt.float16 if OUT_DT16 else F32
    nc = bacc.Bacc("TRN2", target_bir_lowering=False,
                   name="cubic_hermite2d", num_devices=N_CORES,
                   num_swdge_queues=N_SWDGE)
    sig_d = nc.dram_tensor("signal", [NB, H, W], MM_DT, kind="ExternalInput")
    w2_d = nc.dram_tensor("w2p", [P, NY], MM_DT, kind="ExternalInput")
    w2b_d = nc.dram_tensor("w2b", [8, NY], MM_DT, kind="ExternalInput")
    w1_d = nc.dram_tensor("w1p", [P, NX], MM_DT, kind="ExternalInput")
    out_d = nc.dram_tensor("out", [NB, NY, NX], OUT_DT, kind="ExternalOutput")

    wlo1_list = sorted({g[2] for g in g1})  # distinct xs source windows
    # per-bank halves of g1 so step2 PSUM tiles are single-bank
    half1 = [[g for g in g1 if g[1] <= NX // 2], [g for g in g1 if g[0] >= NX // 2]]
    assert sum(map(len, half1)) == len(g1)

    with (
        TileContext(nc) as tc,
        tc.tile_pool(name="const", bufs=1) as const_pool,
        tc.tile_pool(name="vbuf", bufs=int(os.environ.get("CH2D_VGEN", "3"))
                     * len(wlo1_list)) as v_pool,
        tc.tile_pool(name="obuf", bufs=int(os.environ.get("CH2D_OBUF", "8"))) as o_pool,
        tc.tile_pool(name="vps", bufs=VPS_BUFS, space="PSUM") as vps_pool,
        tc.tile_pool(name="ops", bufs=OPS_BUFS, space="PSUM") as ops_pool,
    ):
        # --- load phase: 6 DMAs, critical path first -------------------
        # sync ring:   sig(b=0, all 4 blocks)  w2b  sigb(all batches)
        # scalar ring: w2p  w1p  sig(b=1..NB-1)   [bulk optionally SWDGE]
        sig_all = const_pool.tile([P, NBLK, NB, W], MM_DT, name="sigall")
        sigb = const_pool.tile([8, NBND, NB, W], MM_DT, name="sigb")
        w2_s = const_pool.tile([P, NY], MM_DT, name="w2s")
        w2b_s = const_pool.tile([8, NY], MM_DT, name="w2bs")
        w1_s = const_pool.tile([P, NX], MM_DT, name="w1s")

        nc.sync.dma_start(
            out=sig_all[:, :, 0, :],
            in_=bass.AP(tensor=sig_d, offset=0,
                        ap=[[W, P], [P * W, NBLK], [1, W]]))
        nc.scalar.dma_start(out=w2_s[:], in_=w2_d[:, :])
        nc.sync.dma_start(out=w2b_s[:], in_=w2b_d[:, :])
        for j in range(NBND):
            nc.sync.dma_start(
                out=sigb[:, j, :, :],
                in_=bass.AP(tensor=sig_d, offset=((j + 1) * P - 4) * W,
                            ap=[[W, 8], [H * W, NB], [1, W]]))
        nc.scalar.dma_start(out=w1_s[:], in_=w1_d[:, :])
        bulk_eng = nc.gpsimd if BULK_ENG == "gp" else nc.scalar
        for b in range(1, NB):
            bulk_eng.dma_start(
                out=sig_all[:, :, b, :],
                in_=bass.AP(tensor=sig_d, offset=b * H * W,
                            ap=[[W, P], [P * W, NBLK], [1, W]]))

        # pre-trigger the ACT 'copy' table load during the load phase so it
        # doesn't stall the first real PSUM drain (~1.3us mid-kernel).
        if ACT_PREWARM:
            warm_a = const_pool.tile([P, 8], F32, name="warma")
            warm_b = const_pool.tile([P, 8], F32, name="warmb")
            nc.vector.memset(warm_a[:], 0)
            nc.scalar.copy(out=warm_b[:], in_=warm_a[:])

        if WARMUP_MMS:
            warm = const_pool.tile([P, 512], MM_DT, name="warm")
            nc.vector.memset(warm[:], 0)
            wps = vps_pool.tile([P, NY], F32, name="ps")
            for i in range(WARMUP_MMS):
                nc.tensor.matmul(out=wps[:, :512], lhsT=warm[:, :P],
                                 rhs=warm[:, :512], start=True, stop=True)

        eng_time = [0.0, 0.0]  # [DVE, ACT] modeled queue time (ns)
        copy_i = [0]

        def copy_out(dst, src):
            # split PSUM->SBUF copies between DVE and ACT, greedily
            # balancing modeled queue time (f32 PSUM -> f16 SBUF, incl.
            # ~150ns sem op): DVE 147+1.05*FD ns, ACT 276+0.82*FD ns.
            fd = src.free_size()
            cost = [(120 + fd) / 0.96 + 150, (172 + fd) / 1.2 + 150]
            e = min(range(2), key=lambda j: eng_time[j] + cost[j])
            eng_time[e] += cost[e]
            if e == 0:
                nc.vector.tensor_copy(out=dst, in_=src)
            else:
                nc.scalar.copy(out=dst, in_=src)
            copy_i[0] += 1

        def build_step1(b, v_tiles_all):
            v_tiles = {}
            for wlo in wlo1_list:
                vt = v_pool.tile([P, NY], MM_DT, name="vt")
                if V_COARSE:
                    vps = vps_pool.tile([P, NY], F32, name="ps")
                    for (rs, re, kind, k) in g2a:
                        if kind == 0:
                            lhsT = sig_all[:, k, b, wlo:wlo + P]
                            rhs = w2_s[:, rs:re]
                        else:
                            lhsT = sigb[:, k, b, wlo:wlo + P]
                            rhs = w2b_s[:, rs:re]
                        nc.tensor.matmul(out=vps[:, rs:re], lhsT=lhsT,
                                         rhs=rhs, start=True, stop=True)
                    copy_out(vt[:], vps[:])
                else:
                    for hb in range(2):
                        base = hb * (NY // 2)
                        vps = vps_pool.tile([P, NY // 2], F32, name="ps")
                        for (rs, re, kind, k) in g2a:
                            if rs < base or re > base + NY // 2:
                                continue
                            if kind == 0:
                                lhsT = sig_all[:, k, b, wlo:wlo + P]
                                rhs = w2_s[:, rs:re]
                            else:
                                lhsT = sigb[:, k, b, wlo:wlo + P]
                                rhs = w2b_s[:, rs:re]
                            nc.tensor.matmul(out=vps[:, rs - base:re - base],
                                             lhsT=lhsT, rhs=rhs,
                                             start=True, stop=True)
                        copy_out(vt[:, base:base + NY // 2], vps[:])
                v_tiles[wlo] = vt
            v_tiles_all[b] = v_tiles

        def build_step2_block(b, mi_list, v_tiles):
            # one staging tile + one store covering r-blocks mi_list of b
            np_ = len(mi_list)
            ot = o_pool.tile([P, np_ * NX], OUT_DT, name="ot",
                             padded_shape=[P, 2 * NX])
            for sub, mi in enumerate(mi_list):
                for hb, hgroups in enumerate(half1):
                    if not hgroups:
                        continue
                    base = hb * (NX // 2)
                    ops = ops_pool.tile([P, NX // 2], F32, name="ps")
                    for (qs, qe, wlo) in hgroups:
                        nc.tensor.matmul(
                            out=ops[:, qs - base:qe - base],
                            lhsT=v_tiles[wlo][:, mi * P:(mi + 1) * P],
                            rhs=w1_s[:, qs:qe],
                            start=True, stop=True,
                        )
                    copy_out(ot[:, sub * NX + base:sub * NX + base + NX // 2],
                             ops[:])
            dst = bass.AP(tensor=out_d,
                          offset=b * NY * NX + mi_list[0] * P * NX,
                          ap=[[NX, P], [P * NX, np_], [1, NX]])
            st_eng = nc.gpsimd if (STORE_SPLIT and copy_i[0] % 2) else nc.sync
            st_eng.dma_start(out=dst, in_=ot[:])

        v_all = {}
        # software pipeline at half-batch granularity: the next batch's
        # step1 (PE-heavy, store-free) is interleaved between the two
        # halves of the current batch's step2, smoothing store traffic.
        # The final batch stores per-block so the tail drains 256KB.
        build_step1(0, v_all)
        for b in range(NB):
            if b + 1 < NB:
                for mp in range(2):
                    build_step2_block(b, [2 * mp, 2 * mp + 1], v_all[b])
                build_step1(b + 1, v_all)
                for mp in range(2, 4):
                    build_step2_block(b, [2 * mp, 2 * mp + 1], v_all[b])
            else:
                for mi in range(NY // P):
                    build_step2_block(b, [mi], v_all[b])

    nc.compile()
    return nc


def _prepare(signal, x1, x2, xs, ys):
    """Host-side prep: sorted-order permutations, interp matrices, groups."""
    xs = np.asarray(xs, dtype=np.float32)
    ys = np.asarray(ys, dtype=np.float32)
    perm_x = None
    if np.any(np.diff(xs) < 0):
        perm_x = np.argsort(xs, kind="stable")
        xs = xs[perm_x]
    perm_y = None
    if np.any(np.diff(ys) < 0):
        perm_y = np.argsort(ys, kind="stable")
        ys = ys[perm_y]

    m1, i1 = _interp_matrix(np.asarray(x1, dtype=np.float64), xs)
    m2, i2 = _interp_matrix(np.asarray(x2, dtype=np.float64), ys)
    g1 = _make_groups(i1, W)
    g2a = _make_groups_aligned(i2, H)

    # pack band blocks: rows = the group's source window
    w1p = np.zeros((P, NX), dtype=np.float32)
    for (qs, qe, wlo) in g1:
        w1p[:, qs:qe] = m1[wlo:wlo + P, qs:qe]
    w2p = np.zeros((P, NY), dtype=np.float32)
    w2b = np.zeros((8, NY), dtype=np.float32)
    for (rs, re, kind, k) in g2a:
        if kind == 0:
            w2p[:, rs:re] = m2[k * P:(k + 1) * P, rs:re]
        else:
            w2b[:, rs:re] = m2[(k + 1) * P - 4:(k + 1) * P + 4, rs:re]
    return g1, g2a, w1p, w2p, w2b, perm_x, perm_y


_NC_CACHE = {}


def _run(inputs, trace=False, trace_kwargs=None):
    signal = np.ascontiguousarray(np.asarray(inputs["signal"], dtype=np.float32))
    g1, g2a, w1p, w2p, w2b, perm_x, perm_y = _prepare(
        signal, inputs["x1"], inputs["x2"], inputs["xs"], inputs["ys"])

    mm_dt = _MM_DTS[MM_MODE]
    key = (tuple(g1), tuple(g2a), mm_dt)
    nc = _NC_CACHE.get(key)
    if nc is None:
        nc = _build_nc(g1, g2a, mm_dt)
        _NC_CACHE[key] = nc

    np_dt = mybir.dt.np(mm_dt)
    sig_cast = signal.astype(np_dt) if np_dt != np.float32 else signal
    w1c, w2c, w2bc = (w1p.astype(np_dt), w2p.astype(np_dt),
                      w2b.astype(np_dt))
    in_maps = []
    for c in range(N_CORES):
        in_maps.append({
            "signal": np.ascontiguousarray(sig_cast[c * NB:(c + 1) * NB]),
            "w2p": w2c,
            "w2b": w2bc,
            "w1p": w1c,
        })
    res = run_bass_kernel_spmd(
        nc, in_maps, core_ids=list(range(N_CORES)),
        trace=trace, **(trace_kwargs or {}),
    )
    out = np.concatenate([np.asarray(r["out"], dtype=np.float32)
                          for r in res.results], axis=0)

    # restore original (unsorted) query order if needed
    if perm_y is not None:
        inv = np.empty_like(perm_y)
        inv[perm_y] = np.arange(len(perm_y))
        out = out[:, inv, :]
    if perm_x is not None:
        inv = np.empty_like(perm_x)
        inv[perm_x] = np.arange(len(perm_x))
        out = out[:, :, inv]
    return out, res


def kernel(signal, x1, x2, xs, ys):
    out, _ = _run({"signal": signal, "x1": x1, "x2": x2, "xs": xs, "ys": ys})
    return out


# revision 6
# speedup vs baseline: 1.2479x; 1.2479x over previous
"""Trainium2 Bass kernel for 2D cubic Hermite interpolation (nn_CubicHermite2d).

Math: with x1 = arange(W), x2 = arange(H) (per the problem spec), the whole
op is linear in `signal`:

    result[b, r, q] = sum_{h,w} M2[h, r] * signal[b, h, w] * M1[w, q]

where M1 [W, Nx] / M2 [H, Ny] are 4-banded cubic-Hermite interpolation
matrices built on the host from xs / ys.  Queries are sorted, so greedy
contiguous query groups have source-row bands inside a single 128-row
window -> every output block is ONE K=128 matmul on the PE (no
accumulation, no transposes):

    step 1:  v[wlo][wp, r]  = sig[hlo:+128, wlo:+128].T @ M2[hlo:+128, rs:re]
    step 2:  out[b, rm, q]  = v[wlo][:, rm*128:+128].T @ M1[wlo:+128, qs:qe]

Matmuls run in float16: 1 cyc/row on the PE, FWL fast weight loads, and
half the load bytes; inputs are O(1) randn so fp16 range is a non-issue
(measured ~1.2e-3 scale-relative error vs the fp32 reference).

Load structure: the critical path (w2 + all five batch-0 signal windows +
w1) is HOST-PACKED into two [128, *] bundles, one DMA per HWDGE ring, so
the PE starts ~8us in instead of ~11.5 (each dma_start costs ~0.7us of
serialized issue time on its ring).  The batch 1..NB-1 window loads follow
on the scalar ring; after that the scalar engine issues no DMA, keeping
its full capacity for PSUM->SBUF drains (it also pre-loads the ACT 'copy'
table during the load phase).  All output stores issue from the sync ring.

The build software-pipelines step1(b+1) between the two step2 halves of
batch b; the last batch stores per-r-block so the kernel tail only drains
a single 256KB store.  CH2D_RAWOUT=k stores the last k r-blocks of each
batch directly from PSUM as f32 (skipping the copy engines at the cost of
2x store bytes for those blocks; host converts and merges).

Sharding: data-parallel over batch B=32 across 8 cores (4 batches/core).
"""

import os
import sys

import numpy as np

for _p in ("/root/.axon_site", "/root/.axon_site/_ro/trn_rl_repo",
           "/root/.axon_site/_ro/pypackages", "/opt/trn_rl_repo"):
    if os.path.isdir(_p) and _p not in sys.path:
        sys.path.append(_p)

import concourse.bass as bass
import concourse.mybir as mybir
from concourse import bacc
from concourse.bass_utils import run_bass_kernel_spmd
from concourse.tile import TileContext

# Problem shapes (hardcoded per spec)
B, H, W = 32, 512, 512
NX, NY = 1024, 1024
N_CORES = 8
NB = B // N_CORES  # batches per core

P = 128
F32 = mybir.dt.float32
MM_MODE = os.environ.get("CH2D_DT", "f16")
_MM_DTS = {"f16": mybir.dt.float16, "bf16": mybir.dt.bfloat16,
           "f32r": mybir.dt.float32r, "f32": mybir.dt.float32}
# store the output as f16 and cast to f32 on host: halves the dominant
# store traffic; adds <=2^-11 relative rounding
OUT_DT16 = os.environ.get("CH2D_OUT16", "1") == "1"
VPS_BUFS = int(os.environ.get("CH2D_VPS", "2"))
OPS_BUFS = int(os.environ.get("CH2D_OPS", "4"))
N_SWDGE = int(os.environ.get("CH2D_SWDGE", "4"))
# V_COARSE: one FD=1024 copy per v tile (vs 2x FD=512)
V_COARSE = os.environ.get("CH2D_VCOARSE", "1") == "1"
WARMUP_MMS = int(os.environ.get("CH2D_WARMUP", "0"))
ACT_PREWARM = os.environ.get("CH2D_ACTWARM", "1") == "1"
# bulk (batches 1..NB-1) signal loads issue on: act (scalar HWDGE ring,
# after the critical bundles) | gp (gpsimd SWDGE queues)
BULK_ENG = os.environ.get("CH2D_BULK", "act")
STORE_SPLIT = os.environ.get("CH2D_STORE_SPLIT", "0") == "1"
# store the last RAWOUT r-blocks of each batch directly from PSUM as f32
RAWOUT = int(os.environ.get("CH2D_RAWOUT", "0"))


def _interp_matrix(x0, u):
    """[n, Q] float64 matrix M with (y @ M) == _interp1d(y, x0, slopes, u) of
    the reference (searchsorted bucket, one-sided/averaged Hermite
    tangents)."""
    x0 = np.asarray(x0, dtype=np.float64)
    n = len(x0)
    q = len(u)
    d = np.diff(x0)  # d[j] = x0[j+1] - x0[j]
    m = np.zeros((n, q), dtype=np.float64)
    idx = np.searchsorted(x0[1:-1], u.astype(np.float64))
    dxq = d[idx]
    t = (u.astype(np.float64) - x0[idx]) / dxq
    t2, t3 = t * t, t * t * t
    h00 = 1.0 - 3.0 * t2 + 2.0 * t3
    h10 = (t - 2.0 * t2 + t3) * dxq   # multiplies m[I]
    h01 = 3.0 * t2 - 2.0 * t3
    h11 = (t3 - t2) * dxq             # multiplies m[I+1]
    for k in range(q):
        i = int(idx[k])
        m[i, k] += h00[k]
        m[i + 1, k] += h01[k]
        c = h10[k]  # m[I]: one-sided at 0, averaged interior
        if i == 0:
            m[1, k] += c / d[0]
            m[0, k] -= c / d[0]
        else:
            m[i + 1, k] += 0.5 * c / d[i]
            m[i, k] += 0.5 * c * (1.0 / d[i - 1] - 1.0 / d[i])
            m[i - 1, k] -= 0.5 * c / d[i - 1]
        c = h11[k]  # m[I+1]
        if i + 1 == n - 1:
            m[n - 1, k] += c / d[n - 2]
            m[n - 2, k] -= c / d[n - 2]
        else:
            m[i + 2, k] += 0.5 * c / d[i + 1]
            m[i + 1, k] += 0.5 * c * (1.0 / d[i] - 1.0 / d[i + 1])
            m[i, k] -= 0.5 * c / d[i]
    return m, idx.astype(np.int64)


def _make_groups(idx, n, max_size=512, bank=512):
    """Greedy contiguous query groups; each group's source rows fit a
    128-row window starting at row_lo.  Groups never cross `bank`-multiples
    in query index (PSUM bank boundary).  Returns [(q_start, q_end,
    row_lo)]."""
    qn = len(idx)
    lo = np.maximum(idx - 1, 0)
    hi = np.minimum(idx + 2, n - 1)
    groups = []
    s = 0
    while s < qn:
        row_lo = int(lo[s])
        e = s
        while e < qn:
            if hi[e] - row_lo + 1 > P:
                break
            if e - s >= max_size:
                break
            if e > s and (e % bank) == 0:
                break
            e += 1
        groups.append((s, e, min(row_lo, n - P)))
        s = e
    return groups


def _build_nc(g1, g2, mm_dt):
    MM_DT = mm_dt
    OUT_DT = mybir.dt.float16 if OUT_DT16 else F32
    nc = bacc.Bacc("TRN2", target_bir_lowering=False,
                   name="cubic_hermite2d", num_devices=N_CORES,
                   num_swdge_queues=N_SWDGE)
    wlo1_list = sorted({g[2] for g in g1})  # distinct xs source windows
    wlo2_list = sorted({g[2] for g in g2})  # distinct ys source windows
    nw2 = len(wlo2_list)
    # packed critical-path bundles (host-built):
    #   pka = [w2p (NY) | b0 sig windows 0..ka-1]
    #   pkb = [b0 sig windows ka.. | w1p (NX)]
    ka = (nw2 + 1) // 2
    pka_w = NY + ka * W
    pkb_w = (nw2 - ka) * W + NX
    pka_d = nc.dram_tensor("pka", [P, pka_w], MM_DT, kind="ExternalInput")
    pkb_d = nc.dram_tensor("pkb", [P, pkb_w], MM_DT, kind="ExternalInput")
    sig_d = nc.dram_tensor("signal", [NB, H, W], MM_DT, kind="ExternalInput")
    out_d = nc.dram_tensor("out", [NB, NY, NX], OUT_DT, kind="ExternalOutput")
    if RAWOUT:
        # raw f32 blocks: [b, j, 128, NX] for the last RAWOUT blocks of b
        o32_d = nc.dram_tensor("out32", [NB, RAWOUT, P, NX], F32,
                               kind="ExternalOutput")

    # per-bank halves so PSUM tiles are single-bank
    half1 = [[g for g in g1 if g[1] <= NX // 2], [g for g in g1 if g[0] >= NX // 2]]
    half2 = [[g for g in g2 if g[1] <= NY // 2], [g for g in g2 if g[0] >= NY // 2]]
    assert sum(map(len, half1)) == len(g1) and sum(map(len, half2)) == len(g2)

    with (
        TileContext(nc) as tc,
        tc.tile_pool(name="const", bufs=1) as const_pool,
        tc.tile_pool(name="sigp", bufs=len(wlo2_list)) as sig_pool,
        tc.tile_pool(name="vbuf", bufs=int(os.environ.get("CH2D_VGEN", "3"))
                     * len(wlo1_list)) as v_pool,
        tc.tile_pool(name="obuf", bufs=int(os.environ.get("CH2D_OBUF", "8"))) as o_pool,
        tc.tile_pool(name="vps", bufs=VPS_BUFS, space="PSUM") as vps_pool,
        tc.tile_pool(name="ops", bufs=OPS_BUFS, space="PSUM") as ops_pool,
    ):
        # --- load phase -------------------------------------------------
        # sync ring:   pka (w2 + first b0 windows)  [then stores]
        # scalar ring: pkb (rest of b0 + w1), then bulk window loads
        pka = const_pool.tile([P, pka_w], MM_DT, name="pka")
        pkb = const_pool.tile([P, pkb_w], MM_DT, name="pkb")
        nc.sync.dma_start(out=pka[:], in_=pka_d[:, :])
        nc.scalar.dma_start(out=pkb[:], in_=pkb_d[:, :])
        w2_s = pka[:, 0:NY]
        w1_s = pkb[:, (nw2 - ka) * W:]

        def sig_b0(i, wlo):  # batch-0 slice of ys-window i
            if i < ka:
                return pka[:, NY + i * W + wlo:NY + i * W + wlo + P]
            j = i - ka
            return pkb[:, j * W + wlo:j * W + wlo + P]

        # bulk: batches 1..NB-1 of each window, one strided DMA per window
        bulk_eng = nc.gpsimd if BULK_ENG == "gp" else nc.scalar
        sig_tiles = {}
        for hlo in wlo2_list:
            st = sig_pool.tile([P, NB - 1, W], MM_DT, name="sigt")
            bulk_eng.dma_start(
                out=st[:],
                in_=bass.AP(tensor=sig_d, offset=H * W + hlo * W,
                            ap=[[W, P], [H * W, NB - 1], [1, W]]))
            sig_tiles[hlo] = st

        def sig_lhs(i, hlo, b, wlo):
            if b == 0:
                return sig_b0(i, wlo)
            return sig_tiles[hlo][:, b - 1, wlo:wlo + P]

        # pre-trigger the ACT 'copy' table load during the load phase so it
        # doesn't stall the first real PSUM drain (~1.3us mid-kernel).
        if ACT_PREWARM:
            warm_a = const_pool.tile([P, 8], F32, name="warma")
            warm_b = const_pool.tile([P, 8], F32, name="warmb")
            nc.vector.memset(warm_a[:], 0)
            nc.scalar.copy(out=warm_b[:], in_=warm_a[:])

        if WARMUP_MMS:
            warm = const_pool.tile([P, 512], MM_DT, name="warm")
            nc.vector.memset(warm[:], 0)
            wps = vps_pool.tile([P, NY], F32, name="ps")
            for i in range(WARMUP_MMS):
                nc.tensor.matmul(out=wps[:, :512], lhsT=warm[:, :P],
                                 rhs=warm[:, :512], start=True, stop=True)

        eng_time = [0.0, 0.0]  # [DVE, ACT] modeled queue time (ns)

        def copy_out(dst, src):
            # split PSUM->SBUF copies between DVE and ACT, greedily
            # balancing modeled queue time (f32 PSUM -> f16 SBUF, incl.
            # ~150ns sem op): DVE 147+1.05*FD ns, ACT 276+0.82*FD ns.
            fd = src.free_size()
            cost = [(120 + fd) / 0.96 + 150, (172 + fd) / 1.2 + 150]
            e = min(range(2), key=lambda j: eng_time[j] + cost[j])
            eng_time[e] += cost[e]
            if e == 0:
                nc.vector.tensor_copy(out=dst, in_=src)
            else:
                nc.scalar.copy(out=dst, in_=src)

        widx = {hlo: i for i, hlo in enumerate(wlo2_list)}

        def build_step1(b, v_tiles_all):
            v_tiles = {}
            for wlo in wlo1_list:
                vt = v_pool.tile([P, NY], MM_DT, name="vt")
                if V_COARSE:
                    vps = vps_pool.tile([P, NY], F32, name="ps")
                    for (rs, re, hlo) in g2:
                        nc.tensor.matmul(
                            out=vps[:, rs:re],
                            lhsT=sig_lhs(widx[hlo], hlo, b, wlo),
                            rhs=w2_s[:, rs:re],
                            start=True, stop=True)
                    copy_out(vt[:], vps[:])
                else:
                    for hb, hgroups in enumerate(half2):
                        if not hgroups:
                            continue
                        base = hb * (NY // 2)
                        vps = vps_pool.tile([P, NY // 2], F32, name="ps")
                        for (rs, re, hlo) in hgroups:
                            nc.tensor.matmul(
                                out=vps[:, rs - base:re - base],
                                lhsT=sig_lhs(widx[hlo], hlo, b, wlo),
                                rhs=w2_s[:, rs:re],
                                start=True, stop=True)
                        copy_out(vt[:, base:base + NY // 2], vps[:])
                v_tiles[wlo] = vt
            v_tiles_all[b] = v_tiles

        store_i = [0]

        def fill_block(b, mi, dst_tile, dst_off, v_tiles):
            # step2 matmuls for r-block mi into PSUM, drained to dst_tile
            for hb, hgroups in enumerate(half1):
                if not hgroups:
                    continue
                base = hb * (NX // 2)
                ops = ops_pool.tile([P, NX // 2], F32, name="ps")
                for (qs, qe, wlo) in hgroups:
                    nc.tensor.matmul(
                        out=ops[:, qs - base:qe - base],
                        lhsT=v_tiles[wlo][:, mi * P:(mi + 1) * P],
                        rhs=w1_s[:, qs:qe],
                        start=True, stop=True)
                copy_out(dst_tile[:, dst_off + base:dst_off + base + NX // 2],
                         ops[:])

        def build_step2_block(b, mi_list, v_tiles):
            # one staging tile + one store covering r-blocks mi_list of b
            np_ = len(mi_list)
            ot = o_pool.tile([P, np_ * NX], OUT_DT, name="ot",
                             padded_shape=[P, 2 * NX])
            for sub, mi in enumerate(mi_list):
                fill_block(b, mi, ot, sub * NX, v_tiles)
            dst = bass.AP(tensor=out_d,
                          offset=b * NY * NX + mi_list[0] * P * NX,
                          ap=[[NX, P], [P * NX, np_], [1, NX]])
            store_i[0] += 1
            st_eng = nc.gpsimd if (STORE_SPLIT and store_i[0] % 2) else nc.sync
            st_eng.dma_start(out=dst, in_=ot[:])

        def build_step2_raw(b, mi, j, v_tiles):
            # step2 for r-block mi stored directly from PSUM as f32
            # (one 1-bank PSUM tile + one 256KB store per half)
            for hb, hgroups in enumerate(half1):
                if not hgroups:
                    continue
                base = hb * (NX // 2)
                ops = ops_pool.tile([P, NX // 2], F32, name="ps")
                for (qs, qe, wlo) in hgroups:
                    nc.tensor.matmul(
                        out=ops[:, qs - base:qe - base],
                        lhsT=v_tiles[wlo][:, mi * P:(mi + 1) * P],
                        rhs=w1_s[:, qs:qe],
                        start=True, stop=True)
                dst = bass.AP(tensor=o32_d,
                              offset=(b * RAWOUT + j) * P * NX + base,
                              ap=[[NX, P], [1, NX // 2]])
                nc.sync.dma_start(out=dst, in_=ops[:])

        v_all = {}
        # software pipeline at half-batch granularity: the next batch's
        # step1 (PE-heavy, store-free) is interleaved between the two
        # halves of the current batch's step2, smoothing store traffic.
        # The final batch stores per-block so the tail drains 256KB.
        nmi = NY // P
        raw_set = set(range(nmi - RAWOUT, nmi))
        build_step1(0, v_all)
        for b in range(NB):
            cooked = [mi for mi in range(nmi) if mi not in raw_set]
            if b + 1 < NB:
                nh = len(cooked) // 2
                for mp in range(0, nh, 2):
                    build_step2_block(b, cooked[mp:mp + 2], v_all[b])
                build_step1(b + 1, v_all)
                for mp in range(nh, len(cooked), 2):
                    build_step2_block(b, cooked[mp:mp + 2], v_all[b])
                for j, mi in enumerate(sorted(raw_set)):
                    build_step2_raw(b, mi, j, v_all[b])
            else:
                for mi in cooked:
                    build_step2_block(b, [mi], v_all[b])
                for j, mi in enumerate(sorted(raw_set)):
                    build_step2_raw(b, mi, j, v_all[b])

    nc.compile()
    return nc


def _prepare(signal, x1, x2, xs, ys):
    """Host-side prep: sorted-order permutations, interp matrices, groups."""
    xs = np.asarray(xs, dtype=np.float32)
    ys = np.asarray(ys, dtype=np.float32)
    perm_x = None
    if np.any(np.diff(xs) < 0):
        perm_x = np.argsort(xs, kind="stable")
        xs = xs[perm_x]
    perm_y = None
    if np.any(np.diff(ys) < 0):
        perm_y = np.argsort(ys, kind="stable")
        ys = ys[perm_y]

    m1, i1 = _interp_matrix(np.asarray(x1, dtype=np.float64), xs)
    m2, i2 = _interp_matrix(np.asarray(x2, dtype=np.float64), ys)
    g1 = _make_groups(i1, W)
    g2 = _make_groups(i2, H)

    # pack band blocks: rows = the group's 128-row source window
    w1p = np.zeros((P, NX), dtype=np.float32)
    for (qs, qe, wlo) in g1:
        w1p[:, qs:qe] = m1[wlo:wlo + P, qs:qe]
    w2p = np.zeros((P, NY), dtype=np.float32)
    for (rs, re, hlo) in g2:
        w2p[:, rs:re] = m2[hlo:hlo + P, rs:re]
    return g1, g2, w1p, w2p, perm_x, perm_y


_NC_CACHE = {}


def _run(inputs, trace=False, trace_kwargs=None):
    signal = np.ascontiguousarray(np.asarray(inputs["signal"], dtype=np.float32))
    g1, g2, w1p, w2p, perm_x, perm_y = _prepare(
        signal, inputs["x1"], inputs["x2"], inputs["xs"], inputs["ys"])

    mm_dt = _MM_DTS[MM_MODE]
    key = (tuple(g1), tuple(g2), mm_dt)
    nc = _NC_CACHE.get(key)
    if nc is None:
        nc = _build_nc(g1, g2, mm_dt)
        _NC_CACHE[key] = nc

    np_dt = mybir.dt.np(mm_dt)
    sig_cast = signal.astype(np_dt) if np_dt != np.float32 else signal
    w1c, w2c = w1p.astype(np_dt), w2p.astype(np_dt)

    wlo2_list = sorted({g[2] for g in g2})
    nw2 = len(wlo2_list)
    ka = (nw2 + 1) // 2
    in_maps = []
    for c in range(N_CORES):
        sc = sig_cast[c * NB:(c + 1) * NB]
        b0w = [sc[0, hlo:hlo + P, :] for hlo in wlo2_list]  # [P, W] each
        pka = np.concatenate([w2c] + b0w[:ka], axis=1)
        pkb = np.concatenate(b0w[ka:] + [w1c], axis=1)
        in_maps.append({
            "pka": np.ascontiguousarray(pka),
            "pkb": np.ascontiguousarray(pkb),
            "signal": np.ascontiguousarray(sc),
        })
    res = run_bass_kernel_spmd(
        nc, in_maps, core_ids=list(range(N_CORES)),
        trace=trace, **(trace_kwargs or {}),
    )
    outs = []
    nmi = NY // P
    for r in res.results:
        o = np.asarray(r["out"], dtype=np.float32)   # [NB, NY, NX]
        if RAWOUT:
            o32 = np.asarray(r["out32"])             # [NB, RAWOUT, P, NX]
            o = o.reshape(NB, nmi, P, NX)
            o[:, nmi - RAWOUT:] = o32
            o = o.reshape(NB, NY, NX)
        outs.append(o)
    out = np.concatenate(outs, axis=0)

    # restore original (unsorted) query order if needed
    if perm_y is not None:
        inv = np.empty_like(perm_y)
        inv[perm_y] = np.arange(len(perm_y))
        out = out[:, inv, :]
    if perm_x is not None:
        inv = np.empty_like(perm_x)
        inv[perm_x] = np.arange(len(perm_x))
        out = out[:, :, inv]
    return out, res


def kernel(signal, x1, x2, xs, ys):
    out, _ = _run({"signal": signal, "x1": x1, "x2": x2, "xs": xs, "ys": ys})
    return out
